# revision 2
# baseline (speedup 1.0000x reference)
"""CAGAT MinSum layer v2 (segment-softmax GNN message passing) on 8 TRN2 cores.

Strategy (v2 — Prelu/exp-factorization rewrite of the 76us baseline)
--------------------------------------------------------------------
Same dst-sharded padded-CSR node-row layout as v1: nodes (and their incoming
edges) are partitioned across 8 cores by destination; on each core partition
p / block b holds one node's edges in a run of W_b columns (degree-sorted
blocks, ~7% pad).  No collectives.

Math: raw[e,k] = lrelu(a_k f_src + b_k f_dst + c_k m + d_k) + p_k m, and the
softmax-mean-scatter reduces to out[n] = s8 * sum_k wsum/(zsum+eps) with
z = exp(raw), w = z*f_src.  v2 exploits exp∘max = max∘exp the OTHER way:
  z = exp(lrelu(t)) * exp(p_k m)
so the leaky-relu runs on the Scalar engine's parametric-relu table
(Prelu, alpha=0.2 — same act table set as Exp/Ln, no table reloads), and
exp(p_k m) / exp(p_k m)*f_src*s8 become HOST-precomputed bf16 input planes
(em / emfs; they also kill pad slots exactly: em=emfs=0 there).

Device per head (planes [128, F]):
  PE   : 3 diagonal matmuls (a,b,c on fs/fd/ms) -> one 4-bank PSUM tile
  ACT  : L = Prelu(PSUM + d_k) f32 (full-F, one instr), E = Exp(L) bf16
  DVE  : zw[t] = E * emem[t] (one dual-plane 2x bf16 mult, stride-0 E view)
  Pool : level-0 pair-sum zh = zwA + zwB (single instr via the zone-split
         column layout: each node's run is scattered as pos->h0,h1 zones so
         "first half + second half" is plane-uniform)
  DVE  : level-1 pair-sum zh2, then per-width-group tensor_reduce (heads
         paired into one reduce instr per group) -> zwsum[P,2,H,nb]
Folds (batched, end): rec = exp(-ln(zsum+eps)) on ACT, prod/head-sum on DVE.
DMA triggers ride the otherwise-idle Sync (SP) queue.
"""

import sys

sys.path.insert(0, "/opt/trn_rl_repo")

import numpy as np

N_NODES = 50000
N_EDGES = 1600000
HEADS = 8
N_CORES = 8
P = 128
EPS_DEN = 1e-12

# head whose whole mult/lvl0/lvl1 chain runs on Pool (slack-scheduled)
POOL_CHAIN = frozenset()
# reduce groups: (first_head, n_heads) fused per tensor_reduce call
RGROUPS = ((0, 4), (4, 2), (6, 2))


# ---------------------------------------------------------------- host prep


def _fold_weights(W_proj, b_proj, W_att, b_att, cycle_penalty, min_sum_scaler):
    H = W_proj.shape[0]
    w = W_proj[:, 0].astype(np.float64)
    Wa = W_att.astype(np.float64)
    a = Wa[:, :H] @ w
    b = Wa[:, H : 2 * H] @ w
    c = Wa[:, 2 * H]
    d = (Wa[:, :H] + Wa[:, H : 2 * H]) @ b_proj.astype(np.float64) + b_att.astype(
        np.float64
    )
    p = cycle_penalty.astype(np.float64)
    s8 = float(min_sum_scaler[0]) / HEADS
    return (
        a.astype(np.float32),
        b.astype(np.float32),
        c.astype(np.float32),
        d.astype(np.float32),
        p.astype(np.float32),
        np.float32(s8),
    )


def _build_layout(dst):
    """Node->(core, partition, block) assignment + unified block widths."""
    n = N_NODES
    deg = np.bincount(dst, minlength=n)
    order = np.argsort(-deg, kind="stable")
    npc = (n + N_CORES - 1) // N_CORES
    nb = (npc + P - 1) // P
    pad_n = npc * N_CORES
    nodes_pad = np.full(pad_n, -1, dtype=np.int64)
    nodes_pad[: len(order)] = order
    node_of = nodes_pad.reshape(npc, N_CORES).T  # [8, npc]

    deg_of = np.where(node_of >= 0, deg[np.clip(node_of, 0, n - 1)], 0)
    pad_npc = nb * P
    deg_pad = np.zeros((N_CORES, pad_npc), dtype=np.int64)
    deg_pad[:, :npc] = deg_of
    blk_max = deg_pad.reshape(N_CORES, nb, P).max(axis=(0, 2))  # [nb]
    W = np.maximum(4, ((blk_max + 3) // 4) * 4).astype(np.int64)  # mult of 4
    colbase = np.zeros(nb + 1, dtype=np.int64)
    colbase[1:] = np.cumsum(W)
    F = int(colbase[-1])

    groups = []  # (block_start, count, width, col_offset) in FULL-width units
    i = 0
    while i < nb:
        jx = i
        while jx < nb and W[jx] == W[i]:
            jx += 1
        groups.append((i, jx - i, int(W[i]), int(colbase[i])))
        i = jx
    return deg, order, node_of, nb, W, colbase, F, groups


def _build_planes(node_features, cycle_mask, src, dst, layout, pvals, s8):
    """fs/fd/ms/em/emfs planes with the 2-level zone-split column layout."""
    deg, order, node_of, nb, W, colbase, F, groups = layout
    n = N_NODES
    nf = node_features.astype(np.float32)
    n_em = pvals.shape[0]

    rank = np.empty(n, dtype=np.int64)
    rank[order] = np.arange(n)
    core_of_node = rank % N_CORES
    j_of_node = rank // N_CORES
    part_of_node = j_of_node % P
    block_of_node = j_of_node // P

    key = core_of_node[dst] * (node_of.shape[1] + 1) + j_of_node[dst]
    eorder = np.argsort(key, kind="stable")
    dsts = dst[eorder]
    srcs = src[eorder]
    msks = cycle_mask[eorder]
    first = np.zeros(len(dsts), dtype=bool)
    first[0] = True
    first[1:] = dsts[1:] != dsts[:-1]
    run_start = np.where(first, np.arange(len(dsts)), 0)
    run_start = np.maximum.accumulate(run_start)
    pos = np.arange(len(dsts)) - run_start

    ce = core_of_node[dsts]
    pe = part_of_node[dsts]
    blk = block_of_node[dsts]
    Wb = W[blk]
    h0 = (pos >= Wb // 2).astype(np.int64)
    pos0 = pos - h0 * (Wb // 2)
    h1 = (pos0 >= Wb // 4).astype(np.int64)
    posq = pos0 - h1 * (Wb // 4)
    cole = h0 * (F // 2) + h1 * (F // 4) + colbase[blk] // 4 + posq
    flat = (ce * P + pe) * F + cole

    fs = np.zeros(N_CORES * P * F, dtype=np.float32)
    ms = np.zeros(N_CORES * P * F, dtype=np.float32)
    fs[flat] = nf[srcs]
    ms[flat] = msks
    em = np.zeros((n_em, N_CORES * P * F), dtype=np.float32)
    emfs = np.zeros((n_em, N_CORES * P * F), dtype=np.float32)
    for j in range(n_em):
        ev = np.exp(pvals[j] * msks)
        em[j, flat] = ev
        emfs[j, flat] = ev * nf[srcs] * s8
    fs = fs.reshape(N_CORES, P, F)
    ms = ms.reshape(N_CORES, P, F)
    em = em.reshape(n_em, N_CORES, P, F)
    emfs = emfs.reshape(n_em, N_CORES, P, F)

    # fd: own-node feature expanded; identical in all four zones
    nf_blk = np.zeros((N_CORES, P, nb), dtype=np.float32)
    jj = j_of_node
    nf_blk[core_of_node, jj % P, jj // P] = nf
    fdq = np.zeros((N_CORES, P, F // 4), dtype=np.float32)
    for (b0, cnt, Wg, off) in groups:
        seg = nf_blk[:, :, b0 : b0 + cnt]
        fdq[:, :, off // 4 : off // 4 + cnt * (Wg // 4)] = np.repeat(
            seg, Wg // 4, axis=2
        )
    fd = np.concatenate([fdq, fdq, fdq, fdq], axis=2)  # [8, P, F]

    # emem: [n_em, 8, P, 2, F] with t=0 -> em, t=1 -> emfs
    emem = np.stack([em, emfs], axis=3)  # [n_em, 8, P, 2, F]
    return fs, fd, ms, emem, nf_blk


# ------------------------------------------------------------- numpy checker


def _numpy_device_sim(fs, fd, ms, emem, coef, layout):
    a, b, c, d, p, s8 = coef
    deg, order, node_of, nb, W, colbase, F, groups = layout
    n_em = emem.shape[0]
    outs = []
    for ci in range(N_CORES):
        zsum = np.zeros((P, HEADS, nb), dtype=np.float32)
        wsum = np.zeros((P, HEADS, nb), dtype=np.float32)
        for k in range(HEADS):
            j = k if n_em == HEADS else 0
            t = a[k] * fs[ci] + b[k] * fd[ci] + c[k] * ms[ci] + d[k]
            L = np.where(t >= 0, t, 0.2 * t)
            E = np.exp(L).astype(np.float32)
            z = E * emem[j, ci, :, 0, :]
            w = E * emem[j, ci, :, 1, :]
            # zone-split inverse: level0 + level1 sums then group reduce
            F2, F4 = F // 2, F // 4
            zh = z[:, :F2] + z[:, F2:]
            zh2 = zh[:, :F4] + zh[:, F4:]
            wh = w[:, :F2] + w[:, F2:]
            wh2 = wh[:, :F4] + wh[:, F4:]
            for (b0, cnt, Wg, off) in groups:
                offq, Wq = off // 4, Wg // 4
                zz = zh2[:, offq : offq + cnt * Wq].reshape(P, cnt, Wq)
                ww = wh2[:, offq : offq + cnt * Wq].reshape(P, cnt, Wq)
                zsum[:, k, b0 : b0 + cnt] = zz.sum(axis=2)
                wsum[:, k, b0 : b0 + cnt] = ww.sum(axis=2)
        den = zsum + np.float32(EPS_DEN)
        outs.append((wsum / den).sum(axis=1))  # s8 already folded into emfs
    return outs


def _assemble(outs, layout):
    deg, order, node_of, nb, W, colbase, F, groups = layout
    npc = node_of.shape[1]
    full = np.zeros(N_NODES, dtype=np.float32)
    jj = np.arange(npc)
    for ci in range(N_CORES):
        vals = outs[ci][jj % P, jj // P]
        nodes = node_of[ci]
        m = nodes >= 0
        full[nodes[m]] = vals[m]
    return full


# ------------------------------------------------------------- bass program


def _build_bass(F, nb, groups, coef, n_em):
    import concourse.bass as bass
    import concourse.tile as tile
    from concourse import mybir
    import bass_rust

    def _split_excess_waits(nc, max_waits=1):
        ctr = [0]
        for bb in nc.main_func.blocks:
            new = []
            for ins in bb.instructions:
                si = ins.sync_info
                if si is not None and si.on_wait and len(si.on_wait) > max_waits:
                    waits = list(si.on_wait)
                    si.on_wait = waits[:max_waits]
                    extras = waits[max_waits:]
                    for i in range(0, len(extras), max_waits):
                        ctr[0] += 1
                        nop = mybir.InstNoOp(name=f"waitsplit-{ctr[0]}", ins=[], outs=[])
                        nop.engine = ins.engine
                        nop.sync_info = bass_rust.SyncInfo(
                            on_wait=extras[i : i + max_waits], on_update=[]
                        )
                        nc.register_instruction(nop, overwrite=True)
                        new.append(nop)
                new.append(ins)
            bb.instructions = new

    a, b, c, d, p, s8 = coef
    f32 = mybir.dt.float32
    bf16 = mybir.dt.bfloat16
    Alu = mybir.AluOpType
    Act = mybir.ActivationFunctionType
    F2, F4 = F // 2, F // 4

    nc = bass.Bass("TRN2")
    fs_d = nc.dram_tensor("fs", [P, F], bf16, kind="ExternalInput")
    nfb_d = nc.dram_tensor("nfb", [P, nb], bf16, kind="ExternalInput")
    ms_d = nc.dram_tensor("ms", [P, F], bf16, kind="ExternalInput")
    emem_d = nc.dram_tensor("emem", [P, n_em * 2 * F], bf16, kind="ExternalInput")
    id_d = nc.dram_tensor("ident", [P, P], bf16, kind="ExternalInput")
    out_d = nc.dram_tensor("out", [P, nb], f32, kind="ExternalOutput")

    chunks = []
    off = 0
    while off < F:
        cw = min(512, F - off)
        chunks.append((off, cw))
        off += cw

    with tile.TileContext(nc) as tc:
        with tc.tile_pool(name="pool", bufs=1) as pool, tc.tile_pool(
            name="psum", bufs=2, space="PSUM"
        ) as psum:
            fs = pool.tile([P, F], bf16)
            fd = pool.tile([P, F], bf16)
            ms = pool.tile([P, F], bf16)
            nfb = pool.tile([P, nb], bf16)
            emem = pool.tile([P, n_em, 2, F], bf16)
            ident = pool.tile([P, P], bf16)
            dgs = [pool.tile([P, 3 * P], bf16, name=f"dg{k}") for k in range(HEADS)]

            planes = [fs, fd, ms]
            # first-wave triggers spread across engines (all idle at boot);
            # identity + chunk0 of fs/ms + nfb first, then the rest
            nc.sync.dma_start(out=ident[:], in_=id_d[:])
            nc.sync.dma_start(out=nfb[:], in_=nfb_d[:])
            nc.scalar.dma_start(out=fs[:, 0:512], in_=fs_d[:, 0:512])
            nc.gpsimd.dma_start(out=ms[:, 0:512], in_=ms_d[:, 0:512])
            nc.scalar.dma_start(out=fs[:, 512:F], in_=fs_d[:, 512:F])
            nc.gpsimd.dma_start(out=ms[:, 512:F], in_=ms_d[:, 512:F])
            em_flat = emem[:].rearrange("p j t f -> p (j t f)")
            nc.sync.dma_start(out=em_flat, in_=emem_d[:])

            # diagonal stationaries built on-device: dgs[k][:, cf*P:...] =
            # ident * coef (saves a 768KB DMA that gated the first matmul).
            # Only the first heads' blocks are built up front; the rest are
            # emitted after head 0 so they hide in the DVE pipeline-fill gap.
            def emit_dg(k):
                for cf, cv in enumerate((a[k], b[k], c[k])):
                    nc.vector.tensor_scalar(
                        out=dgs[k][:, cf * P : (cf + 1) * P], in0=ident[:],
                        scalar1=float(cv), scalar2=None, op0=Alu.mult,
                    )

            for k in range(3):
                emit_dg(k)

            # build fd on-device: expand nfb [P, nb] to the zone-split plane
            # (same value in all four zones; stride-0 broadcast copies)
            fd4 = fd[:].rearrange("p (z q) -> p z q", z=4)
            for (b0, cnt, Wg, off) in groups:
                offq, Wq = off // 4, Wg // 4
                src_v = (
                    nfb[:, b0 : b0 + cnt]
                    .unsqueeze(1)
                    .unsqueeze(3)
                    .broadcast_to([P, 4, cnt, Wq])
                )
                dst_v = fd4[:, :, offq : offq + cnt * Wq].rearrange(
                    "p z (c w) -> p z c w", w=Wq
                )
                nc.vector.tensor_copy(out=dst_v, in_=src_v)

            dbias = pool.tile([P, HEADS], f32)
            for k in range(HEADS):
                nc.vector.memset(dbias[:, k : k + 1], float(d[k]))
            eps_b = pool.tile([P, 1], f32)
            nc.vector.memset(eps_b[:], float(EPS_DEN))

            zwsum = pool.tile([P, 2, HEADS, nb], f32)

            import contextlib

            _hstack = contextlib.ExitStack()
            hpool = _hstack.enter_context(tc.tile_pool(name="hpool", bufs=3))
            # paired-reduce staging tiles live across REDUCE_PAIR heads
            prpool = _hstack.enter_context(
                tc.tile_pool(name="prpool", bufs=2)
            )

            acc = pool.tile([P, nb], f32)
            zh_of = {}
            zh2p_of = {}

            def emit_head(k):
                j = k if n_em == HEADS else 0
                tp = psum.tile([P, 2048], f32, tag="tp")
                L = hpool.tile([P, F], f32, tag="L")
                E = hpool.tile([P, F], bf16, tag="E")
                zw = hpool.tile([P, 2, 2, F2], bf16, tag="zw")
                Edup = E[:].unsqueeze(1).broadcast_to([P, 2, F])
                zw_flat = zw[:].rearrange("p t h q -> p t (h q)")

                def mm(off, cw):
                    for cf in range(3):
                        nc.tensor.matmul(
                            tp[:, off : off + cw],
                            lhsT=dgs[k][:, cf * P : (cf + 1) * P],
                            rhs=planes[cf][:, off : off + cw],
                            start=(cf == 0),
                            stop=(cf == 2),
                        )

                eng = nc.gpsimd if k in POOL_CHAIN else nc.vector
                if k <= 1:
                    # fully chunked + interleaved first head: each chunk's
                    # Prelu/Exp/mult is emitted right after its matmuls so the
                    # tile-granular PSUM dependency doesn't wait for chunk 3
                    for (off, cw) in chunks:
                        sl = slice(off, off + cw)
                        mm(off, cw)
                        nc.scalar.activation(
                            out=L[:, sl], in_=tp[:, sl], func=Act.Prelu,
                            bias=dbias[:, k : k + 1], alpha=0.2,
                        )
                        nc.scalar.activation(out=E[:, sl], in_=L[:, sl], func=Act.Exp)
                        nc.vector.tensor_tensor(
                            out=zw_flat[:, :, sl], in0=Edup[:, :, sl],
                            in1=emem[:, j, :, sl], op=Alu.mult,
                        )
                elif k == HEADS - 1:
                    # last head: halved Exp/mult so the tail chain starts early
                    for (off, cw) in chunks:
                        mm(off, cw)
                    nc.scalar.activation(
                        out=L[:], in_=tp[:, 0:F], func=Act.Prelu,
                        bias=dbias[:, k : k + 1], alpha=0.2,
                    )
                    for sl in (slice(0, F2), slice(F2, F)):
                        nc.scalar.activation(out=E[:, sl], in_=L[:, sl], func=Act.Exp)
                        nc.vector.tensor_tensor(
                            out=zw_flat[:, :, sl], in0=Edup[:, :, sl],
                            in1=emem[:, j, :, sl], op=Alu.mult,
                        )
                else:
                    for (off, cw) in chunks:
                        mm(off, cw)
                    nc.scalar.activation(
                        out=L[:], in_=tp[:, 0:F], func=Act.Prelu,
                        bias=dbias[:, k : k + 1], alpha=0.2,
                    )
                    nc.scalar.activation(out=E[:], in_=L[:], func=Act.Exp)
                    if k in POOL_CHAIN:
                        # plain (non-broadcast) mults for the Q7 ucode
                        nc.gpsimd.tensor_tensor(
                            out=zw_flat[:, 0], in0=E[:], in1=emem[:, j, 0],
                            op=Alu.mult,
                        )
                        nc.gpsimd.tensor_tensor(
                            out=zw_flat[:, 1], in0=E[:], in1=emem[:, j, 1],
                            op=Alu.mult,
                        )
                    else:
                        nc.vector.tensor_tensor(
                            out=zw_flat, in0=Edup, in1=emem[:, j], op=Alu.mult
                        )
                # level-0: zh[p,t,h1,r] = zwA + zwB
                zh = hpool.tile([P, 2, 2, F4], bf16, tag="zh")
                zh_flat = zh[:].rearrange("p t h q -> p t (h q)")
                eng.tensor_tensor(
                    out=zh_flat, in0=zw[:, :, 0], in1=zw[:, :, 1], op=Alu.add
                )
                zh_of[k] = zh

            GROUP_OF = {}
            for (g0, gn) in RGROUPS:
                for kk in range(g0, g0 + gn):
                    GROUP_OF[kk] = (g0, gn)

            def emit_lvl1(k):
                g0, gn = GROUP_OF[k]
                if k == g0:
                    zh2p_of[g0] = prpool.tile(
                        [P, 2, gn, F4], bf16, tag=f"zh2p{gn}", name="zh2p"
                    )
                zh2p = zh2p_of[g0]
                zh = zh_of.pop(k)
                eng = nc.gpsimd if k in POOL_CHAIN else nc.vector
                eng.tensor_tensor(
                    out=zh2p[:, :, k - g0],
                    in0=zh[:, :, 0],
                    in1=zh[:, :, 1],
                    op=Alu.add,
                )

            def emit_pair_tail(k):
                # reduces + folds for the group ending at head k
                g0, gn = GROUP_OF[k]
                k0 = g0
                zh2p = zh2p_of.pop(g0)
                for (b0, cnt, Wg, off) in groups:
                    offq, Wq = off // 4, Wg // 4
                    zin = zh2p[:, :, :, offq : offq + cnt * Wq].rearrange(
                        "p t h (c w) -> p t h c w", w=Wq
                    )
                    nc.vector.tensor_reduce(
                        out=zwsum[:, :, k0 : k + 1, b0 : b0 + cnt],
                        in_=zin,
                        axis=mybir.AxisListType.X,
                        op=Alu.add,
                    )
                lg = hpool.tile([P, gn, nb], f32, tag=f"lg{gn}")
                rec = hpool.tile([P, gn, nb], f32, tag=f"rec{gn}")
                prod = hpool.tile([P, gn, nb], f32, tag=f"prod{gn}")
                nc.scalar.activation(
                    out=lg[:].rearrange("p h n -> p (h n)"),
                    in_=zwsum[:, 0, k0 : k + 1].rearrange("p h n -> p (h n)"),
                    func=Act.Ln,
                    bias=eps_b[:],
                )
                nc.scalar.activation(
                    out=rec[:].rearrange("p h n -> p (h n)"),
                    in_=lg[:].rearrange("p h n -> p (h n)"),
                    func=Act.Exp,
                    scale=-1.0,
                )
                nc.vector.tensor_tensor(
                    out=prod[:].rearrange("p h n -> p (h n)"),
                    in0=zwsum[:, 1, k0 : k + 1].rearrange("p h n -> p (h n)"),
                    in1=rec[:].rearrange("p h n -> p (h n)"),
                    op=Alu.mult,
                )
                psum_pair = hpool.tile([P, nb], f32, tag="psum_pair")
                nc.vector.tensor_reduce(
                    out=psum_pair[:],
                    in_=prod[:].rearrange("p h n -> p n h"),
                    axis=mybir.AxisListType.X,
                    op=Alu.add,
                )
                if k0 == 0:
                    nc.vector.tensor_copy(out=acc[:], in_=psum_pair[:])
                else:
                    nc.vector.tensor_add(out=acc[:], in0=acc[:], in1=psum_pair[:])

            group_last = {g0 + gn - 1 for (g0, gn) in RGROUPS}
            for st in range(HEADS + 2):
                if st < HEADS:
                    emit_head(st)
                if st == 0:
                    for kk in range(3, HEADS):
                        emit_dg(kk)
                kl = st - 1
                if 0 <= kl < HEADS:
                    emit_lvl1(kl)
                kp = st - 2
                if kp >= 0 and kp in group_last:
                    emit_pair_tail(kp)

            _hstack.close()
            nc.sync.dma_start(out=out_d[:], in_=acc[:])
    _split_excess_waits(nc)
    return nc


# -------------------------------------------------------------------- kernel

_trace_flag = {"trace": False, "last": None}


def kernel(
    node_features,
    cycle_mask,
    W_proj,
    b_proj,
    W_att,
    b_att,
    cycle_penalty,
    min_sum_scaler,
    edge_index,
    _numpy=False,
):
    node_features = np.asarray(node_features)
    cycle_mask = np.asarray(cycle_mask)
    edge_index = np.asarray(edge_index)
    src = edge_index[0].astype(np.int64)
    dst = edge_index[1].astype(np.int64)

    coef = _fold_weights(
        np.asarray(W_proj), np.asarray(b_proj), np.asarray(W_att),
        np.asarray(b_att), np.asarray(cycle_penalty), np.asarray(min_sum_scaler),
    )
    a, b, c, d, p, s8 = coef
    if np.all(p == p[0]):
        pvals = p[:1]
    else:
        pvals = p
    n_em = pvals.shape[0]

    layout = _build_layout(dst)
    fs, fd, ms, emem, nf_blk = _build_planes(
        node_features, cycle_mask, src, dst, layout, pvals, s8
    )
    deg, order, node_of, nb, W, colbase, F, groups = layout

    if _numpy:
        outs = _numpy_device_sim(fs, fd, ms, emem, coef, layout)
        return _assemble(outs, layout)

    from concourse.bass_utils import run_bass_kernel_spmd

    nc = _build_bass(F, nb, groups, coef, n_em)
    import ml_dtypes

    bf = ml_dtypes.bfloat16
    ident = np.eye(P, dtype=np.float32).astype(bf)
    in_maps = []
    for ci in range(N_CORES):
        m = {
            "fs": fs[ci].astype(bf),
            "nfb": nf_blk[ci].astype(bf),
            "ms": ms[ci].astype(bf),
            "emem": emem[:, ci].transpose(1, 0, 2, 3).reshape(P, n_em * 2 * F).astype(bf),
            "ident": ident,
        }
        in_maps.append(m)
    res = run_bass_kernel_spmd(
        nc, in_maps, core_ids=list(range(N_CORES)), trace=_trace_flag["trace"]
    )
    _trace_flag["last"] = res
    outs = [res.results[ci]["out"] for ci in range(N_CORES)]
    return _assemble(outs, layout)


# revision 3
# speedup vs baseline: 1.0010x; 1.0010x over previous
"""CAGAT MinSum layer (segment-softmax GNN message passing) on 8 TRN2 cores.

Strategy (v2: Prelu/exp-factorization rewrite; 76.2us baseline -> ~62-64us)
---------------------------------------------------------------------------
Sharding: nodes (and their incoming edges) are partitioned across the 8 cores
by destination, so each core owns its output slice and no collective is
needed.  Per core, partition p / block b of the [128, F] planes holds one
node's edges (degree-sorted blocks, per-block padded width W_b, ~7% pad).

Math: raw[e,k] = lrelu(a_k*f_src + b_k*f_dst + c_k*m + d_k) + p_k*m and the
softmax-mean-scatter collapses to out[n] = sum_k wsum_k/(zsum_k+eps) with
z = exp(raw), w = z*f_src*s8.  Two key factorizations:
  * lrelu runs on the Scalar engine's parametric-relu table (Prelu,
    alpha=0.2; same act-table set as Exp/Ln so no table reloads), straight
    out of PSUM: L = Prelu(PSUM + d_k) in ONE full-F instruction (the PSUM
    tile spans 4 banks; matmuls write bank-aligned 512-col chunks).
  * exp(lrelu(t) + p*m) = exp(lrelu(t)) * exp(p*m), so exp(p_k*m) and
    exp(p_k*m)*f_src*s8 are HOST-precomputed bf16 planes (em/emfs,
    interleaved as one emem plane); they also zero pad slots exactly.

Column layout is 2-level zone-split: each node's W_b slots are scattered so
"first half" / "second half" (and quarters) are plane-uniform column ranges.
The dst-segment sums then need only two full-plane bf16 adds (2 elem/cyc)
before a short per-width-group tensor_reduce (1 elem/cyc) on quarter planes,
with several heads fused per reduce instruction.

Per head: PE 3 diagonal matmuls (stationaries built on-device from a DMA'd
identity; fd plane expanded on-device from per-node features via stride-0
broadcast copies) -> ACT Prelu + full-F Exp -> DVE dual-plane mult
(stride-0-duplicated E against emem), two pair-sum adds, grouped reduces.
Folds rec = exp(-ln(zsum+eps)) ride ACT per reduce-group; head-sum + output
accumulate on DVE.  DMA triggers are spread across the Sync/Scalar/GpSimd
queues; heads 0-1 run fully chunk-interleaved (and head 0 chunk 0 reads a
small duplicate em0 tile) so compute starts while planes stream in.

All elementwise/reduce work runs on DVE alone: co-running GpSimd tensor ops
share DVE's SBUF ports and stretch whichever engine starts second by up to
6x (measured), so Pool assistance is a net loss.  DVE is the critical path:
~40us busy, gapless in steady state (mult 1.9us + lvl0 1.0 + lvl1 0.6 +
reduces ~1.1 per head), plus ~14us DMA/pipeline-fill ramp and a ~5us tail.
Measured 62.0-64.9us over 7 runs (run variance +-1.5us), rel err 2.04e-3
(bf16-dominated).
"""

import sys

sys.path.insert(0, "/opt/trn_rl_repo")

import numpy as np

N_NODES = 50000
N_EDGES = 1600000
HEADS = 8
N_CORES = 8
P = 128
EPS_DEN = 1e-12

# head whose whole mult/lvl0/lvl1 chain runs on Pool (slack-scheduled)
POOL_CHAIN = frozenset()
# reduce groups: (first_head, n_heads) fused per tensor_reduce call
RGROUPS = ((0, 4), (4, 2), (6, 1), (7, 1))


# ---------------------------------------------------------------- host prep


def _fold_weights(W_proj, b_proj, W_att, b_att, cycle_penalty, min_sum_scaler):
    H = W_proj.shape[0]
    w = W_proj[:, 0].astype(np.float64)
    Wa = W_att.astype(np.float64)
    a = Wa[:, :H] @ w
    b = Wa[:, H : 2 * H] @ w
    c = Wa[:, 2 * H]
    d = (Wa[:, :H] + Wa[:, H : 2 * H]) @ b_proj.astype(np.float64) + b_att.astype(
        np.float64
    )
    p = cycle_penalty.astype(np.float64)
    s8 = float(min_sum_scaler[0]) / HEADS
    return (
        a.astype(np.float32),
        b.astype(np.float32),
        c.astype(np.float32),
        d.astype(np.float32),
        p.astype(np.float32),
        np.float32(s8),
    )


def _build_layout(dst):
    """Node->(core, partition, block) assignment + unified block widths."""
    n = N_NODES
    deg = np.bincount(dst, minlength=n)
    order = np.argsort(-deg, kind="stable")
    npc = (n + N_CORES - 1) // N_CORES
    nb = (npc + P - 1) // P
    pad_n = npc * N_CORES
    nodes_pad = np.full(pad_n, -1, dtype=np.int64)
    nodes_pad[: len(order)] = order
    node_of = nodes_pad.reshape(npc, N_CORES).T  # [8, npc]

    deg_of = np.where(node_of >= 0, deg[np.clip(node_of, 0, n - 1)], 0)
    pad_npc = nb * P
    deg_pad = np.zeros((N_CORES, pad_npc), dtype=np.int64)
    deg_pad[:, :npc] = deg_of
    blk_max = deg_pad.reshape(N_CORES, nb, P).max(axis=(0, 2))  # [nb]
    W = np.maximum(4, ((blk_max + 3) // 4) * 4).astype(np.int64)  # mult of 4
    colbase = np.zeros(nb + 1, dtype=np.int64)
    colbase[1:] = np.cumsum(W)
    F = int(colbase[-1])

    groups = []  # (block_start, count, width, col_offset) in FULL-width units
    i = 0
    while i < nb:
        jx = i
        while jx < nb and W[jx] == W[i]:
            jx += 1
        groups.append((i, jx - i, int(W[i]), int(colbase[i])))
        i = jx
    return deg, order, node_of, nb, W, colbase, F, groups


def _build_planes(node_features, cycle_mask, src, dst, layout, pvals, s8):
    """fs/fd/ms/em/emfs planes with the 2-level zone-split column layout."""
    deg, order, node_of, nb, W, colbase, F, groups = layout
    n = N_NODES
    nf = node_features.astype(np.float32)
    n_em = pvals.shape[0]

    rank = np.empty(n, dtype=np.int64)
    rank[order] = np.arange(n)
    core_of_node = rank % N_CORES
    j_of_node = rank // N_CORES
    part_of_node = j_of_node % P
    block_of_node = j_of_node // P

    key = core_of_node[dst] * (node_of.shape[1] + 1) + j_of_node[dst]
    eorder = np.argsort(key, kind="stable")
    dsts = dst[eorder]
    srcs = src[eorder]
    msks = cycle_mask[eorder]
    first = np.zeros(len(dsts), dtype=bool)
    first[0] = True
    first[1:] = dsts[1:] != dsts[:-1]
    run_start = np.where(first, np.arange(len(dsts)), 0)
    run_start = np.maximum.accumulate(run_start)
    pos = np.arange(len(dsts)) - run_start

    ce = core_of_node[dsts]
    pe = part_of_node[dsts]
    blk = block_of_node[dsts]
    Wb = W[blk]
    h0 = (pos >= Wb // 2).astype(np.int64)
    pos0 = pos - h0 * (Wb // 2)
    h1 = (pos0 >= Wb // 4).astype(np.int64)
    posq = pos0 - h1 * (Wb // 4)
    cole = h0 * (F // 2) + h1 * (F // 4) + colbase[blk] // 4 + posq
    flat = (ce * P + pe) * F + cole

    fs = np.zeros(N_CORES * P * F, dtype=np.float32)
    ms = np.zeros(N_CORES * P * F, dtype=np.float32)
    fs[flat] = nf[srcs]
    ms[flat] = msks
    em = np.zeros((n_em, N_CORES * P * F), dtype=np.float32)
    emfs = np.zeros((n_em, N_CORES * P * F), dtype=np.float32)
    for j in range(n_em):
        ev = np.exp(pvals[j] * msks)
        em[j, flat] = ev
        emfs[j, flat] = ev * nf[srcs] * s8
    fs = fs.reshape(N_CORES, P, F)
    ms = ms.reshape(N_CORES, P, F)
    em = em.reshape(n_em, N_CORES, P, F)
    emfs = emfs.reshape(n_em, N_CORES, P, F)

    # fd: own-node feature expanded; identical in all four zones
    nf_blk = np.zeros((N_CORES, P, nb), dtype=np.float32)
    jj = j_of_node
    nf_blk[core_of_node, jj % P, jj // P] = nf
    fdq = np.zeros((N_CORES, P, F // 4), dtype=np.float32)
    for (b0, cnt, Wg, off) in groups:
        seg = nf_blk[:, :, b0 : b0 + cnt]
        fdq[:, :, off // 4 : off // 4 + cnt * (Wg // 4)] = np.repeat(
            seg, Wg // 4, axis=2
        )
    fd = np.concatenate([fdq, fdq, fdq, fdq], axis=2)  # [8, P, F]

    # emem: [n_em, 8, P, 2, F] with t=0 -> em, t=1 -> emfs
    emem = np.stack([em, emfs], axis=3)  # [n_em, 8, P, 2, F]
    return fs, fd, ms, emem, nf_blk


# ------------------------------------------------------------- numpy checker


def _numpy_device_sim(fs, fd, ms, emem, coef, layout):
    a, b, c, d, p, s8 = coef
    deg, order, node_of, nb, W, colbase, F, groups = layout
    n_em = emem.shape[0]
    outs = []
    for ci in range(N_CORES):
        zsum = np.zeros((P, HEADS, nb), dtype=np.float32)
        wsum = np.zeros((P, HEADS, nb), dtype=np.float32)
        for k in range(HEADS):
            j = k if n_em == HEADS else 0
            t = a[k] * fs[ci] + b[k] * fd[ci] + c[k] * ms[ci] + d[k]
            L = np.where(t >= 0, t, 0.2 * t)
            E = np.exp(L).astype(np.float32)
            z = E * emem[j, ci, :, 0, :]
            w = E * emem[j, ci, :, 1, :]
            # zone-split inverse: level0 + level1 sums then group reduce
            F2, F4 = F // 2, F // 4
            zh = z[:, :F2] + z[:, F2:]
            zh2 = zh[:, :F4] + zh[:, F4:]
            wh = w[:, :F2] + w[:, F2:]
            wh2 = wh[:, :F4] + wh[:, F4:]
            for (b0, cnt, Wg, off) in groups:
                offq, Wq = off // 4, Wg // 4
                zz = zh2[:, offq : offq + cnt * Wq].reshape(P, cnt, Wq)
                ww = wh2[:, offq : offq + cnt * Wq].reshape(P, cnt, Wq)
                zsum[:, k, b0 : b0 + cnt] = zz.sum(axis=2)
                wsum[:, k, b0 : b0 + cnt] = ww.sum(axis=2)
        den = zsum + np.float32(EPS_DEN)
        outs.append((wsum / den).sum(axis=1))  # s8 already folded into emfs
    return outs


def _assemble(outs, layout):
    deg, order, node_of, nb, W, colbase, F, groups = layout
    npc = node_of.shape[1]
    full = np.zeros(N_NODES, dtype=np.float32)
    jj = np.arange(npc)
    for ci in range(N_CORES):
        vals = outs[ci][jj % P, jj // P]
        nodes = node_of[ci]
        m = nodes >= 0
        full[nodes[m]] = vals[m]
    return full


# ------------------------------------------------------------- bass program


def _build_bass(F, nb, groups, coef, n_em):
    import concourse.bass as bass
    import concourse.tile as tile
    from concourse import mybir
    import bass_rust

    def _split_excess_waits(nc, max_waits=1):
        ctr = [0]
        for bb in nc.main_func.blocks:
            new = []
            for ins in bb.instructions:
                si = ins.sync_info
                if si is not None and si.on_wait and len(si.on_wait) > max_waits:
                    waits = list(si.on_wait)
                    si.on_wait = waits[:max_waits]
                    extras = waits[max_waits:]
                    for i in range(0, len(extras), max_waits):
                        ctr[0] += 1
                        nop = mybir.InstNoOp(name=f"waitsplit-{ctr[0]}", ins=[], outs=[])
                        nop.engine = ins.engine
                        nop.sync_info = bass_rust.SyncInfo(
                            on_wait=extras[i : i + max_waits], on_update=[]
                        )
                        nc.register_instruction(nop, overwrite=True)
                        new.append(nop)
                new.append(ins)
            bb.instructions = new

    a, b, c, d, p, s8 = coef
    f32 = mybir.dt.float32
    bf16 = mybir.dt.bfloat16
    Alu = mybir.AluOpType
    Act = mybir.ActivationFunctionType
    F2, F4 = F // 2, F // 4

    nc = bass.Bass("TRN2")
    fs_d = nc.dram_tensor("fs", [P, F], bf16, kind="ExternalInput")
    nfb_d = nc.dram_tensor("nfb", [P, nb], bf16, kind="ExternalInput")
    ms_d = nc.dram_tensor("ms", [P, F], bf16, kind="ExternalInput")
    emem_d = nc.dram_tensor("emem", [P, n_em * 2 * F], bf16, kind="ExternalInput")
    em0_d = nc.dram_tensor("em0", [P, 2, 512], bf16, kind="ExternalInput")
    id_d = nc.dram_tensor("ident", [P, P], bf16, kind="ExternalInput")
    out_d = nc.dram_tensor("out", [P, nb], f32, kind="ExternalOutput")

    chunks = []
    off = 0
    while off < F:
        cw = min(512, F - off)
        chunks.append((off, cw))
        off += cw

    with tile.TileContext(nc) as tc:
        with tc.tile_pool(name="pool", bufs=1) as pool, tc.tile_pool(
            name="psum", bufs=2, space="PSUM"
        ) as psum:
            fs = pool.tile([P, F], bf16)
            fd = pool.tile([P, F], bf16)
            ms = pool.tile([P, F], bf16)
            nfb = pool.tile([P, nb], bf16)
            emem = pool.tile([P, n_em, 2, F], bf16)
            em0 = pool.tile([P, 2, 512], bf16)
            ident = pool.tile([P, P], bf16)
            dgs = [pool.tile([P, 3 * P], bf16, name=f"dg{k}") for k in range(HEADS)]

            planes = [fs, fd, ms]
            # first-wave triggers spread across engines (all idle at boot);
            # identity + chunk0 of fs/ms + nfb first, then the rest
            nc.sync.dma_start(out=nfb[:], in_=nfb_d[:])
            nc.sync.dma_start(out=ident[:], in_=id_d[:])
            nc.sync.dma_start(out=em0[:], in_=em0_d[:])
            nc.scalar.dma_start(out=fs[:, 0:512], in_=fs_d[:, 0:512])
            nc.gpsimd.dma_start(out=ms[:, 0:512], in_=ms_d[:, 0:512])
            nc.scalar.dma_start(out=fs[:, 512:F], in_=fs_d[:, 512:F])
            nc.gpsimd.dma_start(out=ms[:, 512:F], in_=ms_d[:, 512:F])
            em_flat = emem[:].rearrange("p j t f -> p (j t f)")
            nc.sync.dma_start(out=em_flat, in_=emem_d[:])

            # diagonal stationaries built on-device: dgs[k][:, cf*P:...] =
            # ident * coef (saves a 768KB DMA that gated the first matmul).
            # Only the first heads' blocks are built up front; the rest are
            # emitted after head 0 so they hide in the DVE pipeline-fill gap.
            def emit_dg(k):
                for cf, cv in enumerate((a[k], b[k], c[k])):
                    nc.vector.tensor_scalar(
                        out=dgs[k][:, cf * P : (cf + 1) * P], in0=ident[:],
                        scalar1=float(cv), scalar2=None, op0=Alu.mult,
                    )

            for k in range(3):
                emit_dg(k)

            # build fd on-device: expand nfb [P, nb] to the zone-split plane
            # (same value in all four zones; stride-0 broadcast copies)
            fd4 = fd[:].rearrange("p (z q) -> p z q", z=4)
            for (b0, cnt, Wg, off) in groups:
                offq, Wq = off // 4, Wg // 4
                src_v = (
                    nfb[:, b0 : b0 + cnt]
                    .unsqueeze(1)
                    .unsqueeze(3)
                    .broadcast_to([P, 4, cnt, Wq])
                )
                dst_v = fd4[:, :, offq : offq + cnt * Wq].rearrange(
                    "p z (c w) -> p z c w", w=Wq
                )
                nc.vector.tensor_copy(out=dst_v, in_=src_v)

            dbias = pool.tile([P, HEADS], f32)
            for k in range(HEADS):
                nc.vector.memset(dbias[:, k : k + 1], float(d[k]))
            eps_b = pool.tile([P, 1], f32)
            nc.vector.memset(eps_b[:], float(EPS_DEN))

            zwsum = pool.tile([P, 2, HEADS, nb], f32)

            import contextlib

            _hstack = contextlib.ExitStack()
            hpool = _hstack.enter_context(tc.tile_pool(name="hpool", bufs=3))
            # paired-reduce staging tiles live across REDUCE_PAIR heads
            prpool = _hstack.enter_context(
                tc.tile_pool(name="prpool", bufs=2)
            )

            acc = pool.tile([P, nb], f32)
            zh_of = {}
            zh2p_of = {}

            def emit_head(k):
                j = k if n_em == HEADS else 0
                tp = psum.tile([P, 2048], f32, tag="tp")
                L = hpool.tile([P, F], f32, tag="L")
                E = hpool.tile([P, F], bf16, tag="E")
                zw = hpool.tile([P, 2, 2, F2], bf16, tag="zw")
                Edup = E[:].unsqueeze(1).broadcast_to([P, 2, F])
                zw_flat = zw[:].rearrange("p t h q -> p t (h q)")

                def mm(off, cw):
                    for i, cf in enumerate((0, 2, 1)):
                        nc.tensor.matmul(
                            tp[:, off : off + cw],
                            lhsT=dgs[k][:, cf * P : (cf + 1) * P],
                            rhs=planes[cf][:, off : off + cw],
                            start=(i == 0),
                            stop=(i == 2),
                        )

                eng = nc.gpsimd if k in POOL_CHAIN else nc.vector
                if k <= 1:
                    # fully chunked + interleaved first head: each chunk's
                    # Prelu/Exp/mult is emitted right after its matmuls so the
                    # tile-granular PSUM dependency doesn't wait for chunk 3
                    for (off, cw) in chunks:
                        sl = slice(off, off + cw)
                        mm(off, cw)
                        nc.scalar.activation(
                            out=L[:, sl], in_=tp[:, sl], func=Act.Prelu,
                            bias=dbias[:, k : k + 1], alpha=0.2,
                        )
                        nc.scalar.activation(out=E[:, sl], in_=L[:, sl], func=Act.Exp)
                        in1 = em0[:] if off == 0 else emem[:, j, :, sl]
                        nc.vector.tensor_tensor(
                            out=zw_flat[:, :, sl], in0=Edup[:, :, sl],
                            in1=in1, op=Alu.mult,
                        )
                elif k == HEADS - 1:
                    # last head: halved Exp/mult so the tail chain starts early
                    for (off, cw) in chunks:
                        mm(off, cw)
                    nc.scalar.activation(
                        out=L[:], in_=tp[:, 0:F], func=Act.Prelu,
                        bias=dbias[:, k : k + 1], alpha=0.2,
                    )
                    for sl in (slice(0, F2), slice(F2, F)):
                        nc.scalar.activation(out=E[:, sl], in_=L[:, sl], func=Act.Exp)
                        nc.vector.tensor_tensor(
                            out=zw_flat[:, :, sl], in0=Edup[:, :, sl],
                            in1=emem[:, j, :, sl], op=Alu.mult,
                        )
                else:
                    for (off, cw) in chunks:
                        mm(off, cw)
                    nc.scalar.activation(
                        out=L[:], in_=tp[:, 0:F], func=Act.Prelu,
                        bias=dbias[:, k : k + 1], alpha=0.2,
                    )
                    nc.scalar.activation(out=E[:], in_=L[:], func=Act.Exp)
                    if k in POOL_CHAIN:
                        # plain (non-broadcast) mults for the Q7 ucode
                        nc.gpsimd.tensor_tensor(
                            out=zw_flat[:, 0], in0=E[:], in1=emem[:, j, 0],
                            op=Alu.mult,
                        )
                        nc.gpsimd.tensor_tensor(
                            out=zw_flat[:, 1], in0=E[:], in1=emem[:, j, 1],
                            op=Alu.mult,
                        )
                    else:
                        nc.vector.tensor_tensor(
                            out=zw_flat, in0=Edup, in1=emem[:, j], op=Alu.mult
                        )
                # level-0: zh[p,t,h1,r] = zwA + zwB
                zh = hpool.tile([P, 2, 2, F4], bf16, tag="zh")
                zh_flat = zh[:].rearrange("p t h q -> p t (h q)")
                eng.tensor_tensor(
                    out=zh_flat, in0=zw[:, :, 0], in1=zw[:, :, 1], op=Alu.add
                )
                zh_of[k] = zh

            GROUP_OF = {}
            for (g0, gn) in RGROUPS:
                for kk in range(g0, g0 + gn):
                    GROUP_OF[kk] = (g0, gn)

            def emit_lvl1(k):
                g0, gn = GROUP_OF[k]
                if k == g0:
                    zh2p_of[g0] = prpool.tile(
                        [P, 2, gn, F4], bf16, tag=f"zh2p{gn}", name="zh2p"
                    )
                zh2p = zh2p_of[g0]
                zh = zh_of.pop(k)
                eng = nc.gpsimd if k in POOL_CHAIN else nc.vector
                eng.tensor_tensor(
                    out=zh2p[:, :, k - g0],
                    in0=zh[:, :, 0],
                    in1=zh[:, :, 1],
                    op=Alu.add,
                )

            def emit_pair_tail(k):
                # reduces + folds for the group ending at head k
                g0, gn = GROUP_OF[k]
                k0 = g0
                zh2p = zh2p_of.pop(g0)
                for (b0, cnt, Wg, off) in groups:
                    offq, Wq = off // 4, Wg // 4
                    zin = zh2p[:, :, :, offq : offq + cnt * Wq].rearrange(
                        "p t h (c w) -> p t h c w", w=Wq
                    )
                    nc.vector.tensor_reduce(
                        out=zwsum[:, :, k0 : k + 1, b0 : b0 + cnt],
                        in_=zin,
                        axis=mybir.AxisListType.X,
                        op=Alu.add,
                    )
                lg = hpool.tile([P, gn, nb], f32, tag=f"lg{gn}")
                rec = hpool.tile([P, gn, nb], f32, tag=f"rec{gn}")
                prod = hpool.tile([P, gn, nb], f32, tag=f"prod{gn}")
                nc.scalar.activation(
                    out=lg[:].rearrange("p h n -> p (h n)"),
                    in_=zwsum[:, 0, k0 : k + 1].rearrange("p h n -> p (h n)"),
                    func=Act.Ln,
                    bias=eps_b[:],
                )
                nc.scalar.activation(
                    out=rec[:].rearrange("p h n -> p (h n)"),
                    in_=lg[:].rearrange("p h n -> p (h n)"),
                    func=Act.Exp,
                    scale=-1.0,
                )
                nc.vector.tensor_tensor(
                    out=prod[:].rearrange("p h n -> p (h n)"),
                    in0=zwsum[:, 1, k0 : k + 1].rearrange("p h n -> p (h n)"),
                    in1=rec[:].rearrange("p h n -> p (h n)"),
                    op=Alu.mult,
                )
                psum_pair = hpool.tile([P, nb], f32, tag="psum_pair")
                nc.vector.tensor_reduce(
                    out=psum_pair[:],
                    in_=prod[:].rearrange("p h n -> p n h"),
                    axis=mybir.AxisListType.X,
                    op=Alu.add,
                )
                if k0 == 0:
                    nc.vector.tensor_copy(out=acc[:], in_=psum_pair[:])
                else:
                    nc.vector.tensor_add(out=acc[:], in0=acc[:], in1=psum_pair[:])

            group_last = {g0 + gn - 1 for (g0, gn) in RGROUPS}
            for st in range(HEADS + 2):
                if st < HEADS:
                    emit_head(st)
                if st == 0:
                    for kk in range(3, HEADS):
                        emit_dg(kk)
                kl = st - 1
                if 0 <= kl < HEADS:
                    emit_lvl1(kl)
                kp = st - 2
                if kp >= 0 and kp in group_last:
                    emit_pair_tail(kp)

            _hstack.close()
            nc.sync.dma_start(out=out_d[:], in_=acc[:])
    _split_excess_waits(nc)
    return nc


# -------------------------------------------------------------------- kernel

_trace_flag = {"trace": False, "last": None}


def kernel(
    node_features,
    cycle_mask,
    W_proj,
    b_proj,
    W_att,
    b_att,
    cycle_penalty,
    min_sum_scaler,
    edge_index,
    _numpy=False,
):
    node_features = np.asarray(node_features)
    cycle_mask = np.asarray(cycle_mask)
    edge_index = np.asarray(edge_index)
    src = edge_index[0].astype(np.int64)
    dst = edge_index[1].astype(np.int64)

    coef = _fold_weights(
        np.asarray(W_proj), np.asarray(b_proj), np.asarray(W_att),
        np.asarray(b_att), np.asarray(cycle_penalty), np.asarray(min_sum_scaler),
    )
    a, b, c, d, p, s8 = coef
    if np.all(p == p[0]):
        pvals = p[:1]
    else:
        pvals = p
    n_em = pvals.shape[0]

    layout = _build_layout(dst)
    fs, fd, ms, emem, nf_blk = _build_planes(
        node_features, cycle_mask, src, dst, layout, pvals, s8
    )
    deg, order, node_of, nb, W, colbase, F, groups = layout

    if _numpy:
        outs = _numpy_device_sim(fs, fd, ms, emem, coef, layout)
        return _assemble(outs, layout)

    from concourse.bass_utils import run_bass_kernel_spmd

    nc = _build_bass(F, nb, groups, coef, n_em)
    import ml_dtypes

    bf = ml_dtypes.bfloat16
    ident = np.eye(P, dtype=np.float32).astype(bf)
    in_maps = []
    for ci in range(N_CORES):
        m = {
            "fs": fs[ci].astype(bf),
            "nfb": nf_blk[ci].astype(bf),
            "ms": ms[ci].astype(bf),
            "emem": emem[:, ci].transpose(1, 0, 2, 3).reshape(P, n_em * 2 * F).astype(bf),
            "em0": emem[0, ci, :, :, 0:512].astype(bf),
            "ident": ident,
        }
        in_maps.append(m)
    res = run_bass_kernel_spmd(
        nc, in_maps, core_ids=list(range(N_CORES)), trace=_trace_flag["trace"]
    )
    _trace_flag["last"] = res
    outs = [res.results[ci]["out"] for ci in range(N_CORES)]
    return _assemble(outs, layout)


# revision 4
# speedup vs baseline: 1.0068x; 1.0057x over previous
"""CAGAT MinSum layer (segment-softmax GNN message passing) on 8 TRN2 cores.

Strategy (v2: Prelu/exp-factorization rewrite; 76.2us baseline -> ~62-64us)
---------------------------------------------------------------------------
Sharding: nodes (and their incoming edges) are partitioned across the 8 cores
by destination, so each core owns its output slice and no collective is
needed.  Per core, partition p / block b of the [128, F] planes holds one
node's edges (degree-sorted blocks, per-block padded width W_b, ~7% pad).

Math: raw[e,k] = lrelu(a_k*f_src + b_k*f_dst + c_k*m + d_k) + p_k*m and the
softmax-mean-scatter collapses to out[n] = sum_k wsum_k/(zsum_k+eps) with
z = exp(raw), w = z*f_src*s8.  Two key factorizations:
  * lrelu runs on the Scalar engine's parametric-relu table (Prelu,
    alpha=0.2; same act-table set as Exp/Ln so no table reloads), straight
    out of PSUM: L = Prelu(PSUM + d_k) in ONE full-F instruction (the PSUM
    tile spans 4 banks; matmuls write bank-aligned 512-col chunks).
  * exp(lrelu(t) + p*m) = exp(lrelu(t)) * exp(p*m), so exp(p_k*m) and
    exp(p_k*m)*f_src*s8 are HOST-precomputed bf16 planes (em/emfs,
    interleaved as one emem plane); they also zero pad slots exactly.

Column layout is 2-level zone-split: each node's W_b slots are scattered so
"first half" / "second half" (and quarters) are plane-uniform column ranges.
The dst-segment sums then need only two full-plane bf16 adds (2 elem/cyc)
before a short per-width-group tensor_reduce (1 elem/cyc) on quarter planes,
with several heads fused per reduce instruction.

Per head: PE 3 diagonal matmuls (stationaries built on-device from a DMA'd
identity; fd plane expanded on-device from per-node features via stride-0
broadcast copies) -> ACT Prelu + full-F Exp -> DVE dual-plane mult
(stride-0-duplicated E against emem), two pair-sum adds, grouped reduces.
Folds rec = exp(-ln(zsum+eps)) ride ACT per reduce-group; each group's
partial output DMAs out as it completes (host sums the four partials), so
only the last group's short fold chain sits on the tail.  DMA triggers are spread across the Sync/Scalar/GpSimd
queues; heads 0-1 run fully chunk-interleaved (and head 0 chunk 0 reads a
small duplicate em0 tile) so compute starts while planes stream in.

All elementwise/reduce work runs on DVE alone: co-running GpSimd tensor ops
share DVE's SBUF ports and stretch whichever engine starts second by up to
6x (measured), so Pool assistance is a net loss.  DVE is the critical path:
~40us busy, gapless in steady state (mult 1.9us + lvl0 1.0 + lvl1 0.6 +
reduces ~1.1 per head), plus ~14us DMA/pipeline-fill ramp and a ~5us tail.
Measured 61.9-63.7us (run variance +-1.5us), rel err 2.04e-3
(bf16-dominated).
"""

import sys

sys.path.insert(0, "/opt/trn_rl_repo")

import numpy as np

N_NODES = 50000
N_EDGES = 1600000
HEADS = 8
N_CORES = 8
P = 128
EPS_DEN = 1e-12

# head whose whole mult/lvl0/lvl1 chain runs on Pool (slack-scheduled)
POOL_CHAIN = frozenset()
# reduce groups: (first_head, n_heads) fused per tensor_reduce call
RGROUPS = ((0, 4), (4, 2), (6, 1), (7, 1))


# ---------------------------------------------------------------- host prep


def _fold_weights(W_proj, b_proj, W_att, b_att, cycle_penalty, min_sum_scaler):
    H = W_proj.shape[0]
    w = W_proj[:, 0].astype(np.float64)
    Wa = W_att.astype(np.float64)
    a = Wa[:, :H] @ w
    b = Wa[:, H : 2 * H] @ w
    c = Wa[:, 2 * H]
    d = (Wa[:, :H] + Wa[:, H : 2 * H]) @ b_proj.astype(np.float64) + b_att.astype(
        np.float64
    )
    p = cycle_penalty.astype(np.float64)
    s8 = float(min_sum_scaler[0]) / HEADS
    return (
        a.astype(np.float32),
        b.astype(np.float32),
        c.astype(np.float32),
        d.astype(np.float32),
        p.astype(np.float32),
        np.float32(s8),
    )


def _build_layout(dst):
    """Node->(core, partition, block) assignment + unified block widths."""
    n = N_NODES
    deg = np.bincount(dst, minlength=n)
    order = np.argsort(-deg, kind="stable")
    npc = (n + N_CORES - 1) // N_CORES
    nb = (npc + P - 1) // P
    pad_n = npc * N_CORES
    nodes_pad = np.full(pad_n, -1, dtype=np.int64)
    nodes_pad[: len(order)] = order
    node_of = nodes_pad.reshape(npc, N_CORES).T  # [8, npc]

    deg_of = np.where(node_of >= 0, deg[np.clip(node_of, 0, n - 1)], 0)
    pad_npc = nb * P
    deg_pad = np.zeros((N_CORES, pad_npc), dtype=np.int64)
    deg_pad[:, :npc] = deg_of
    blk_max = deg_pad.reshape(N_CORES, nb, P).max(axis=(0, 2))  # [nb]
    W = np.maximum(4, ((blk_max + 3) // 4) * 4).astype(np.int64)  # mult of 4
    colbase = np.zeros(nb + 1, dtype=np.int64)
    colbase[1:] = np.cumsum(W)
    F = int(colbase[-1])

    groups = []  # (block_start, count, width, col_offset) in FULL-width units
    i = 0
    while i < nb:
        jx = i
        while jx < nb and W[jx] == W[i]:
            jx += 1
        groups.append((i, jx - i, int(W[i]), int(colbase[i])))
        i = jx
    return deg, order, node_of, nb, W, colbase, F, groups


def _build_planes(node_features, cycle_mask, src, dst, layout, pvals, s8):
    """fs/fd/ms/em/emfs planes with the 2-level zone-split column layout."""
    deg, order, node_of, nb, W, colbase, F, groups = layout
    n = N_NODES
    nf = node_features.astype(np.float32)
    n_em = pvals.shape[0]

    rank = np.empty(n, dtype=np.int64)
    rank[order] = np.arange(n)
    core_of_node = rank % N_CORES
    j_of_node = rank // N_CORES
    part_of_node = j_of_node % P
    block_of_node = j_of_node // P

    key = core_of_node[dst] * (node_of.shape[1] + 1) + j_of_node[dst]
    eorder = np.argsort(key, kind="stable")
    dsts = dst[eorder]
    srcs = src[eorder]
    msks = cycle_mask[eorder]
    first = np.zeros(len(dsts), dtype=bool)
    first[0] = True
    first[1:] = dsts[1:] != dsts[:-1]
    run_start = np.where(first, np.arange(len(dsts)), 0)
    run_start = np.maximum.accumulate(run_start)
    pos = np.arange(len(dsts)) - run_start

    ce = core_of_node[dsts]
    pe = part_of_node[dsts]
    blk = block_of_node[dsts]
    Wb = W[blk]
    h0 = (pos >= Wb // 2).astype(np.int64)
    pos0 = pos - h0 * (Wb // 2)
    h1 = (pos0 >= Wb // 4).astype(np.int64)
    posq = pos0 - h1 * (Wb // 4)
    cole = h0 * (F // 2) + h1 * (F // 4) + colbase[blk] // 4 + posq
    flat = (ce * P + pe) * F + cole

    fs = np.zeros(N_CORES * P * F, dtype=np.float32)
    ms = np.zeros(N_CORES * P * F, dtype=np.float32)
    fs[flat] = nf[srcs]
    ms[flat] = msks
    em = np.zeros((n_em, N_CORES * P * F), dtype=np.float32)
    emfs = np.zeros((n_em, N_CORES * P * F), dtype=np.float32)
    for j in range(n_em):
        ev = np.exp(pvals[j] * msks)
        em[j, flat] = ev
        emfs[j, flat] = ev * nf[srcs] * s8
    fs = fs.reshape(N_CORES, P, F)
    ms = ms.reshape(N_CORES, P, F)
    em = em.reshape(n_em, N_CORES, P, F)
    emfs = emfs.reshape(n_em, N_CORES, P, F)

    # fd: own-node feature expanded; identical in all four zones
    nf_blk = np.zeros((N_CORES, P, nb), dtype=np.float32)
    jj = j_of_node
    nf_blk[core_of_node, jj % P, jj // P] = nf
    fdq = np.zeros((N_CORES, P, F // 4), dtype=np.float32)
    for (b0, cnt, Wg, off) in groups:
        seg = nf_blk[:, :, b0 : b0 + cnt]
        fdq[:, :, off // 4 : off // 4 + cnt * (Wg // 4)] = np.repeat(
            seg, Wg // 4, axis=2
        )
    fd = np.concatenate([fdq, fdq, fdq, fdq], axis=2)  # [8, P, F]

    # emem: [n_em, 8, P, 2, F] with t=0 -> em, t=1 -> emfs
    emem = np.stack([em, emfs], axis=3)  # [n_em, 8, P, 2, F]
    return fs, fd, ms, emem, nf_blk


# ------------------------------------------------------------- numpy checker


def _numpy_device_sim(fs, fd, ms, emem, coef, layout):
    a, b, c, d, p, s8 = coef
    deg, order, node_of, nb, W, colbase, F, groups = layout
    n_em = emem.shape[0]
    outs = []
    for ci in range(N_CORES):
        zsum = np.zeros((P, HEADS, nb), dtype=np.float32)
        wsum = np.zeros((P, HEADS, nb), dtype=np.float32)
        for k in range(HEADS):
            j = k if n_em == HEADS else 0
            t = a[k] * fs[ci] + b[k] * fd[ci] + c[k] * ms[ci] + d[k]
            L = np.where(t >= 0, t, 0.2 * t)
            E = np.exp(L).astype(np.float32)
            z = E * emem[j, ci, :, 0, :]
            w = E * emem[j, ci, :, 1, :]
            # zone-split inverse: level0 + level1 sums then group reduce
            F2, F4 = F // 2, F // 4
            zh = z[:, :F2] + z[:, F2:]
            zh2 = zh[:, :F4] + zh[:, F4:]
            wh = w[:, :F2] + w[:, F2:]
            wh2 = wh[:, :F4] + wh[:, F4:]
            for (b0, cnt, Wg, off) in groups:
                offq, Wq = off // 4, Wg // 4
                zz = zh2[:, offq : offq + cnt * Wq].reshape(P, cnt, Wq)
                ww = wh2[:, offq : offq + cnt * Wq].reshape(P, cnt, Wq)
                zsum[:, k, b0 : b0 + cnt] = zz.sum(axis=2)
                wsum[:, k, b0 : b0 + cnt] = ww.sum(axis=2)
        den = zsum + np.float32(EPS_DEN)
        outs.append((wsum / den).sum(axis=1))  # s8 already folded into emfs
    return outs


def _assemble(outs, layout):
    deg, order, node_of, nb, W, colbase, F, groups = layout
    npc = node_of.shape[1]
    full = np.zeros(N_NODES, dtype=np.float32)
    jj = np.arange(npc)
    for ci in range(N_CORES):
        vals = outs[ci][jj % P, jj // P]
        nodes = node_of[ci]
        m = nodes >= 0
        full[nodes[m]] = vals[m]
    return full


# ------------------------------------------------------------- bass program


def _build_bass(F, nb, groups, coef, n_em):
    import concourse.bass as bass
    import concourse.tile as tile
    from concourse import mybir
    import bass_rust

    def _split_excess_waits(nc, max_waits=1):
        ctr = [0]
        for bb in nc.main_func.blocks:
            new = []
            for ins in bb.instructions:
                si = ins.sync_info
                if si is not None and si.on_wait and len(si.on_wait) > max_waits:
                    waits = list(si.on_wait)
                    si.on_wait = waits[:max_waits]
                    extras = waits[max_waits:]
                    for i in range(0, len(extras), max_waits):
                        ctr[0] += 1
                        nop = mybir.InstNoOp(name=f"waitsplit-{ctr[0]}", ins=[], outs=[])
                        nop.engine = ins.engine
                        nop.sync_info = bass_rust.SyncInfo(
                            on_wait=extras[i : i + max_waits], on_update=[]
                        )
                        nc.register_instruction(nop, overwrite=True)
                        new.append(nop)
                new.append(ins)
            bb.instructions = new

    a, b, c, d, p, s8 = coef
    f32 = mybir.dt.float32
    bf16 = mybir.dt.bfloat16
    Alu = mybir.AluOpType
    Act = mybir.ActivationFunctionType
    F2, F4 = F // 2, F // 4

    nc = bass.Bass("TRN2")
    fs_d = nc.dram_tensor("fs", [P, F], bf16, kind="ExternalInput")
    nfb_d = nc.dram_tensor("nfb", [P, nb], bf16, kind="ExternalInput")
    ms_d = nc.dram_tensor("ms", [P, F], bf16, kind="ExternalInput")
    emem_d = nc.dram_tensor("emem", [P, n_em * 2 * F], bf16, kind="ExternalInput")
    em0_d = nc.dram_tensor("em0", [P, 2, 512], bf16, kind="ExternalInput")
    id_d = nc.dram_tensor("ident", [P, P], bf16, kind="ExternalInput")
    ngroups = len(RGROUPS)
    out_d = nc.dram_tensor("out", [P, ngroups * nb], f32, kind="ExternalOutput")

    chunks = []
    off = 0
    while off < F:
        cw = min(512, F - off)
        chunks.append((off, cw))
        off += cw

    with tile.TileContext(nc) as tc:
        with tc.tile_pool(name="pool", bufs=1) as pool, tc.tile_pool(
            name="psum", bufs=2, space="PSUM"
        ) as psum:
            fs = pool.tile([P, F], bf16)
            fd = pool.tile([P, F], bf16)
            ms = pool.tile([P, F], bf16)
            nfb = pool.tile([P, nb], bf16)
            emem = pool.tile([P, n_em, 2, F], bf16)
            em0 = pool.tile([P, 2, 512], bf16)
            ident = pool.tile([P, P], bf16)
            dgs = [pool.tile([P, 3 * P], bf16, name=f"dg{k}") for k in range(HEADS)]

            planes = [fs, fd, ms]
            # first-wave triggers spread across engines (all idle at boot);
            # identity + chunk0 of fs/ms + nfb first, then the rest
            nc.sync.dma_start(out=nfb[:], in_=nfb_d[:])
            nc.sync.dma_start(out=ident[:], in_=id_d[:])
            nc.sync.dma_start(out=em0[:], in_=em0_d[:])
            nc.scalar.dma_start(out=fs[:, 0:512], in_=fs_d[:, 0:512])
            nc.gpsimd.dma_start(out=ms[:, 0:512], in_=ms_d[:, 0:512])
            nc.scalar.dma_start(out=fs[:, 512:F], in_=fs_d[:, 512:F])
            nc.gpsimd.dma_start(out=ms[:, 512:F], in_=ms_d[:, 512:F])
            em_flat = emem[:].rearrange("p j t f -> p (j t f)")
            nc.sync.dma_start(out=em_flat, in_=emem_d[:])

            # diagonal stationaries built on-device: dgs[k][:, cf*P:...] =
            # ident * coef (saves a 768KB DMA that gated the first matmul).
            # Only the first heads' blocks are built up front; the rest are
            # emitted after head 0 so they hide in the DVE pipeline-fill gap.
            def emit_dg(k):
                for cf, cv in enumerate((a[k], b[k], c[k])):
                    nc.vector.tensor_scalar(
                        out=dgs[k][:, cf * P : (cf + 1) * P], in0=ident[:],
                        scalar1=float(cv), scalar2=None, op0=Alu.mult,
                    )

            for k in range(3):
                emit_dg(k)

            # build fd on-device: expand nfb [P, nb] to the zone-split plane
            # (same value in all four zones; stride-0 broadcast copies)
            fd4 = fd[:].rearrange("p (z q) -> p z q", z=4)
            for (b0, cnt, Wg, off) in groups:
                offq, Wq = off // 4, Wg // 4
                src_v = (
                    nfb[:, b0 : b0 + cnt]
                    .unsqueeze(1)
                    .unsqueeze(3)
                    .broadcast_to([P, 4, cnt, Wq])
                )
                dst_v = fd4[:, :, offq : offq + cnt * Wq].rearrange(
                    "p z (c w) -> p z c w", w=Wq
                )
                nc.vector.tensor_copy(out=dst_v, in_=src_v)

            dbias = pool.tile([P, HEADS], f32)
            for k in range(HEADS):
                nc.vector.memset(dbias[:, k : k + 1], float(d[k]))
            eps_b = pool.tile([P, 1], f32)
            nc.vector.memset(eps_b[:], float(EPS_DEN))

            zwsum = pool.tile([P, 2, HEADS, nb], f32)

            import contextlib

            _hstack = contextlib.ExitStack()
            hpool = _hstack.enter_context(tc.tile_pool(name="hpool", bufs=3))
            # paired-reduce staging tiles live across REDUCE_PAIR heads
            prpool = _hstack.enter_context(
                tc.tile_pool(name="prpool", bufs=2)
            )

            zh_of = {}
            zh2p_of = {}

            def emit_head(k):
                j = k if n_em == HEADS else 0
                tp = psum.tile([P, 2048], f32, tag="tp")
                L = hpool.tile([P, F], f32, tag="L")
                E = hpool.tile([P, F], bf16, tag="E")
                zw = hpool.tile([P, 2, 2, F2], bf16, tag="zw")
                Edup = E[:].unsqueeze(1).broadcast_to([P, 2, F])
                zw_flat = zw[:].rearrange("p t h q -> p t (h q)")

                def mm(off, cw):
                    for i, cf in enumerate((0, 2, 1)):
                        nc.tensor.matmul(
                            tp[:, off : off + cw],
                            lhsT=dgs[k][:, cf * P : (cf + 1) * P],
                            rhs=planes[cf][:, off : off + cw],
                            start=(i == 0),
                            stop=(i == 2),
                        )

                eng = nc.gpsimd if k in POOL_CHAIN else nc.vector
                if k <= 1:
                    # fully chunked + interleaved first head: each chunk's
                    # Prelu/Exp/mult is emitted right after its matmuls so the
                    # tile-granular PSUM dependency doesn't wait for chunk 3
                    for (off, cw) in chunks:
                        sl = slice(off, off + cw)
                        mm(off, cw)
                        nc.scalar.activation(
                            out=L[:, sl], in_=tp[:, sl], func=Act.Prelu,
                            bias=dbias[:, k : k + 1], alpha=0.2,
                        )
                        nc.scalar.activation(out=E[:, sl], in_=L[:, sl], func=Act.Exp)
                        in1 = em0[:] if off == 0 else emem[:, j, :, sl]
                        nc.vector.tensor_tensor(
                            out=zw_flat[:, :, sl], in0=Edup[:, :, sl],
                            in1=in1, op=Alu.mult,
                        )
                elif k == HEADS - 1:
                    # last head: halved Exp/mult so the tail chain starts early
                    for (off, cw) in chunks:
                        mm(off, cw)
                    nc.scalar.activation(
                        out=L[:], in_=tp[:, 0:F], func=Act.Prelu,
                        bias=dbias[:, k : k + 1], alpha=0.2,
                    )
                    for sl in (slice(0, F2), slice(F2, F)):
                        nc.scalar.activation(out=E[:, sl], in_=L[:, sl], func=Act.Exp)
                        nc.vector.tensor_tensor(
                            out=zw_flat[:, :, sl], in0=Edup[:, :, sl],
                            in1=emem[:, j, :, sl], op=Alu.mult,
                        )
                else:
                    for (off, cw) in chunks:
                        mm(off, cw)
                    nc.scalar.activation(
                        out=L[:], in_=tp[:, 0:F], func=Act.Prelu,
                        bias=dbias[:, k : k + 1], alpha=0.2,
                    )
                    nc.scalar.activation(out=E[:], in_=L[:], func=Act.Exp)
                    if k in POOL_CHAIN:
                        # plain (non-broadcast) mults for the Q7 ucode
                        nc.gpsimd.tensor_tensor(
                            out=zw_flat[:, 0], in0=E[:], in1=emem[:, j, 0],
                            op=Alu.mult,
                        )
                        nc.gpsimd.tensor_tensor(
                            out=zw_flat[:, 1], in0=E[:], in1=emem[:, j, 1],
                            op=Alu.mult,
                        )
                    else:
                        nc.vector.tensor_tensor(
                            out=zw_flat, in0=Edup, in1=emem[:, j], op=Alu.mult
                        )
                # level-0: zh[p,t,h1,r] = zwA + zwB
                zh = hpool.tile([P, 2, 2, F4], bf16, tag="zh")
                zh_flat = zh[:].rearrange("p t h q -> p t (h q)")
                eng.tensor_tensor(
                    out=zh_flat, in0=zw[:, :, 0], in1=zw[:, :, 1], op=Alu.add
                )
                zh_of[k] = zh

            GROUP_OF = {}
            for (g0, gn) in RGROUPS:
                for kk in range(g0, g0 + gn):
                    GROUP_OF[kk] = (g0, gn)

            def emit_lvl1(k):
                g0, gn = GROUP_OF[k]
                if k == g0:
                    zh2p_of[g0] = prpool.tile(
                        [P, 2, gn, F4], bf16, tag=f"zh2p{gn}", name="zh2p"
                    )
                zh2p = zh2p_of[g0]
                zh = zh_of.pop(k)
                eng = nc.gpsimd if k in POOL_CHAIN else nc.vector
                eng.tensor_tensor(
                    out=zh2p[:, :, k - g0],
                    in0=zh[:, :, 0],
                    in1=zh[:, :, 1],
                    op=Alu.add,
                )

            def emit_pair_tail(k):
                # reduces + folds for the group ending at head k
                g0, gn = GROUP_OF[k]
                k0 = g0
                zh2p = zh2p_of.pop(g0)
                for (b0, cnt, Wg, off) in groups:
                    offq, Wq = off // 4, Wg // 4
                    zin = zh2p[:, :, :, offq : offq + cnt * Wq].rearrange(
                        "p t h (c w) -> p t h c w", w=Wq
                    )
                    nc.vector.tensor_reduce(
                        out=zwsum[:, :, k0 : k + 1, b0 : b0 + cnt],
                        in_=zin,
                        axis=mybir.AxisListType.X,
                        op=Alu.add,
                    )
                lg = hpool.tile([P, gn, nb], f32, tag=f"lg{gn}")
                rec = hpool.tile([P, gn, nb], f32, tag=f"rec{gn}")
                prod = hpool.tile([P, gn, nb], f32, tag=f"prod{gn}")
                nc.scalar.activation(
                    out=lg[:].rearrange("p h n -> p (h n)"),
                    in_=zwsum[:, 0, k0 : k + 1].rearrange("p h n -> p (h n)"),
                    func=Act.Ln,
                    bias=eps_b[:],
                )
                nc.scalar.activation(
                    out=rec[:].rearrange("p h n -> p (h n)"),
                    in_=lg[:].rearrange("p h n -> p (h n)"),
                    func=Act.Exp,
                    scale=-1.0,
                )
                nc.vector.tensor_tensor(
                    out=prod[:].rearrange("p h n -> p (h n)"),
                    in0=zwsum[:, 1, k0 : k + 1].rearrange("p h n -> p (h n)"),
                    in1=rec[:].rearrange("p h n -> p (h n)"),
                    op=Alu.mult,
                )
                gi = [g[0] for g in RGROUPS].index(g0)
                osl = slice(gi * nb, (gi + 1) * nb)
                if gn == 1:
                    nc.sync.dma_start(out=out_d[:, osl], in_=prod[:, 0])
                else:
                    psum_pair = hpool.tile([P, nb], f32, tag="psum_pair")
                    nc.vector.tensor_reduce(
                        out=psum_pair[:],
                        in_=prod[:].rearrange("p h n -> p n h"),
                        axis=mybir.AxisListType.X,
                        op=Alu.add,
                    )
                    nc.sync.dma_start(out=out_d[:, osl], in_=psum_pair[:])

            group_last = {g0 + gn - 1 for (g0, gn) in RGROUPS}
            for st in range(HEADS + 2):
                if st < HEADS:
                    emit_head(st)
                if st == 0:
                    for kk in range(3, HEADS):
                        emit_dg(kk)
                kl = st - 1
                if 0 <= kl < HEADS:
                    emit_lvl1(kl)
                kp = st - 2
                if kp >= 0 and kp in group_last:
                    emit_pair_tail(kp)

            _hstack.close()
    _split_excess_waits(nc)
    return nc


# -------------------------------------------------------------------- kernel

_trace_flag = {"trace": False, "last": None}


def kernel(
    node_features,
    cycle_mask,
    W_proj,
    b_proj,
    W_att,
    b_att,
    cycle_penalty,
    min_sum_scaler,
    edge_index,
    _numpy=False,
):
    node_features = np.asarray(node_features)
    cycle_mask = np.asarray(cycle_mask)
    edge_index = np.asarray(edge_index)
    src = edge_index[0].astype(np.int64)
    dst = edge_index[1].astype(np.int64)

    coef = _fold_weights(
        np.asarray(W_proj), np.asarray(b_proj), np.asarray(W_att),
        np.asarray(b_att), np.asarray(cycle_penalty), np.asarray(min_sum_scaler),
    )
    a, b, c, d, p, s8 = coef
    if np.all(p == p[0]):
        pvals = p[:1]
    else:
        pvals = p
    n_em = pvals.shape[0]

    layout = _build_layout(dst)
    fs, fd, ms, emem, nf_blk = _build_planes(
        node_features, cycle_mask, src, dst, layout, pvals, s8
    )
    deg, order, node_of, nb, W, colbase, F, groups = layout

    if _numpy:
        outs = _numpy_device_sim(fs, fd, ms, emem, coef, layout)
        return _assemble(outs, layout)

    from concourse.bass_utils import run_bass_kernel_spmd

    nc = _build_bass(F, nb, groups, coef, n_em)
    import ml_dtypes

    bf = ml_dtypes.bfloat16
    ident = np.eye(P, dtype=np.float32).astype(bf)
    in_maps = []
    for ci in range(N_CORES):
        m = {
            "fs": fs[ci].astype(bf),
            "nfb": nf_blk[ci].astype(bf),
            "ms": ms[ci].astype(bf),
            "emem": emem[:, ci].transpose(1, 0, 2, 3).reshape(P, n_em * 2 * F).astype(bf),
            "em0": emem[0, ci, :, :, 0:512].astype(bf),
            "ident": ident,
        }
        in_maps.append(m)
    res = run_bass_kernel_spmd(
        nc, in_maps, core_ids=list(range(N_CORES)), trace=_trace_flag["trace"]
    )
    _trace_flag["last"] = res
    ng = len(RGROUPS)
    outs = [
        res.results[ci]["out"].reshape(P, ng, nb).sum(axis=1) for ci in range(N_CORES)
    ]
    return _assemble(outs, layout)
